# revision 4
# baseline (speedup 1.0000x reference)
"""Multi-head attention on 8 Trainium2 NeuronCores (head-parallel), v3.

Problem: Q,K,V [4096,512] fp32; Wq/Wk/Wv [8,512,64]; Wo [512,512].
  out = concat_h(softmax(QWq_h (KWk_h)^T / sqrt(64)) VWv_h) @ Wo

Sharding: one head per core; each core also computes its slice of the
output projection (out_h @ Wo[64h:64h+64, :]); the host sums the 8
partial [4096,512] outputs.

Numerics (float32r matmuls only carry ~14 mantissa bits on this target,
so the near-fp32 paths are built from fp16/fp8 pieces):
  proj   q,k in 3-term fp16 hi/lo (Q = Qh+Ql, W = Wh+Wl, drop lolo,
         ~22-bit); v single-pass fp16.
  stats  per-row score max from the fp16 hi parts (shift-invariance of
         softmax cancels the ~+-3 estimate error); DVE
         tensor_tensor_reduce folds two 512-key chunks per op.
  main   transposed scores = fp16 hi.hi matmul with a 65th contraction
         row (k side -1, q side rowmax) so PSUM gets s - rowmax, plus
         fp8e4m3 DoubleRow matmuls carrying both cross terms
         (lo operands pre-scaled by 2^8/2^7 into e4m3 range).
  att.V  fp16 attn (est window ~e^4) x fp16 v; a ones column in v emits
         the softmax denominator row.
  wo     f32r out-projection (final linear layer tolerates 14-bit),
         scaled by 1/denominator on eviction.
"""

from contextlib import ExitStack

import numpy as np

N = 4096
DIM = 512
H = 8
D = 64
P = 128
CH = 512   # queries per era
NCH = N // CH
NT = N // P      # 32 key tiles
KC = DIM // P    # 4 contraction chunks for projections
NB = N // 512    # 8 input column blocks
SK = 256.0       # fp8 scaling, slot0: (k_lo*SK) . (q_hi/SK)
SQ = 128.0       # fp8 scaling, slot1: (k_hi/SQ) . (q_lo*SQ)


def build_head_kernel(ctx, tc, outs, ins, n=N, dim=DIM, d=D):
    import concourse.bass as bass
    import concourse.mybir as mybir
    from concourse.bass import ts, ds

    nc = tc.nc
    f32 = mybir.dt.float32
    f32r = mybir.dt.float32r
    fp16 = mybir.dt.float16
    f8e4 = mybir.dt.float8e4
    AF = mybir.ActivationFunctionType
    AX = mybir.AxisListType
    DR = mybir.MatmulPerfMode.DoubleRow

    qth_d, qtl_d = ins["QTH"], ins["QTL"]
    kth_d, ktl_d = ins["KTH"], ins["KTL"]
    vt_d = ins["VT"]
    wo_d = ins["wo"]
    out_d = outs["out"]

    singles = ctx.enter_context(tc.tile_pool(name="singles", bufs=1))

    A = singles.tile([d + 1, n], fp16)     # q16; row d = rowmax est (fp16)
    B = singles.tile([d + 1, n], fp16)     # k16; row d = -1
    Q8 = singles.tile([d, 2, n], f8e4)     # slots: (q16l*SQ, q16/SK)
    K8 = singles.tile([d, 2, n], f8e4)     # slots: (k16/SQ, k16l*SK)
    v_sb = singles.tile([P, NT, d + 1], fp16)  # v tiles + ones column
    outT = singles.tile([d, n], f32r)      # attn_u @ v (unnormalized)
    sumx = singles.tile([1, CH], f32)      # denominators (per era)
    rsum = singles.tile([P, NT], f32)
    rinv = singles.tile([P, NT], f32)
    wqh_sb = singles.tile([P, KC, d], fp16)
    wql_sb = singles.tile([P, KC, d], fp16)
    wkh_sb = singles.tile([P, KC, d], fp16)
    wkl_sb = singles.tile([P, KC, d], fp16)
    wv_sb = singles.tile([P, KC, d], fp16)
    wo_sb = singles.tile([d, dim], f32r)

    # PSUM pools: nat 2x[128,2x512] = 4 banks (shared with wo/qproj),
    # main 3x[128,512] = 3, av 1.
    nat_pool = ctx.enter_context(tc.tile_pool(name="nat_ps", bufs=2, space="PSUM"))
    wo_pool = nat_pool
    att_pool = ctx.enter_context(tc.tile_pool(name="att", bufs=6))
    nmax_pool = ctx.enter_context(tc.tile_pool(name="nmax", bufs=5))
    o_pool = ctx.enter_context(tc.tile_pool(name="osb", bufs=4))
    qt_hold = ctx.enter_context(tc.tile_pool(name="qt_hold", bufs=4))

    nmax_tiles = {}

    def nat_item(c, jt, j):
        """Stats for era-c q-tile jt: key chunk j (1024 keys as two 512
        halves); one tensor_tensor_reduce folds both halves."""
        qs = ts(c * 4 + jt, P)
        st = nat_pool.tile([P, 2, 512], f32, tag="nat")
        nc.tensor.matmul(st[:, 0, :], lhsT=A[0:d, qs], rhs=B[0:d, ts(2 * j, 512)],
                         start=True, stop=True)
        nc.tensor.matmul(st[:, 1, :], lhsT=A[0:d, qs], rhs=B[0:d, ts(2 * j + 1, 512)],
                         start=True, stop=True)
        if j == 0:
            nmax_tiles[jt] = nmax_pool.tile([P, 5], fp16, tag="nmax", name="nmax")
        nm = nmax_tiles[jt]
        nc.vector.reduce_max(nm[:, j:j + 1], st, axis=AX.XY)
        if j == 3:
            nc.vector.reduce_max(nm[:, 4:5], nm[:, 0:4], axis=AX.X)
            nc.sync.dma_start(out=A[d:d + 1, ds(c * CH + jt * P, P)],
                              in_=nm[:, 4:5])

    def stats_items(c):
        """16 stats items for era c (4 q-tiles x 4 key chunks)."""
        return [lambda c=c, jt=jt, j=j: nat_item(c, jt, j)
                for j in range(4) for jt in range(4)]

    # ---- P1: load + projections, era-0 stats folded in ----
    pending = []

    def flush(k=1):
        for _ in range(k):
            if pending:
                pending.pop(0)()

    qt_blocks = {}

    def hold_q(nb):
        th = qt_hold.tile([P, KC, 512], fp16, tag="qtbh", name=f"qtbh{nb}")
        nc.sync.dma_start(out=th, in_=qth_d[:, ds(nb * 512, 512)]
                          .rearrange("(c p) x -> p c x", p=P))
        tl = qt_hold.tile([P, KC, 512], fp16, tag="qtbl", name=f"qtbl{nb}")
        nc.sync.dma_start(out=tl, in_=qtl_d[:, ds(nb * 512, 512)]
                          .rearrange("(c p) x -> p c x", p=P))
        qt_blocks[nb] = (th, tl)

    def qk_proj(nb, src_h, src_l, dst, dst8, wh, wl, hi_slot, s_hi, s_lo, ps):
        """3-term fp16 projection + fp16/fp8 operand prep.  dst rows 0:d
        get the fp16 hi; dst8 slot hi_slot = hi*s_hi, the other = lo*s_lo,
        so the DoubleRow matmul pairs k_hi.q_lo and k_lo.q_hi."""
        nbs = ds(nb * 512, 512)
        terms = [(wh, src_h), (wh, src_l), (wl, src_h)]
        for i, (w, x) in enumerate(terms):
            for kc in range(KC):
                nc.tensor.matmul(ps, lhsT=w[:, kc, :], rhs=x[:, kc, :],
                                 start=(i == 0 and kc == 0),
                                 stop=(i == 2 and kc == KC - 1))
        nc.scalar.copy(dst[0:d, nbs], ps)
        lo = o_pool.tile([d, 512], fp16, tag="lo", name="lo")
        nc.vector.tensor_sub(lo, ps, dst[0:d, nbs])
        nc.scalar.mul(dst8[:, hi_slot, nbs], dst[0:d, nbs], float(s_hi))
        nc.vector.tensor_scalar_mul(dst8[:, 1 - hi_slot, nbs], lo, float(s_lo))

    with tc.tile_pool(name="qk_stream", bufs=3) as qk_stream, \
         tc.tile_pool(name="v_stream", bufs=2) as v_stream, \
         tc.tile_pool(name="pq_ps", bufs=1, space="PSUM") as pq_pool, \
         tc.tile_pool(name="pv_ps", bufs=2, space="PSUM") as pv_pool:

        def load_block(t_d, tag, pool=None):
            pool = pool or qk_stream
            t = pool.tile([P, KC, 512], fp16, tag=tag, name=tag)
            nc.sync.dma_start(out=t, in_=t_d.rearrange("(c p) x -> p c x", p=P))
            return t

        def v_proj(nb, vt_t):
            ps = pv_pool.tile([P, 4, d], f32, tag="pv")
            for i in range(4):
                for kc in range(KC):
                    nc.tensor.matmul(ps[:, i, :], lhsT=vt_t[:, kc, ts(i, P)],
                                     rhs=wv_sb[:, kc, :],
                                     start=(kc == 0), stop=(kc == KC - 1))
                flush()
            nc.vector.tensor_copy(v_sb[:, ds(4 * nb, 4), 0:d], ps)

        # DMA order: weights, K0-7 (hi+lo), Q0, V0-7, Q1-7
        nc.vector.memset(B[d:d + 1, :], -1.0)
        nc.vector.memset(v_sb[:, :, d:d + 1], 1.0)
        for name, sb in (("wkh", wkh_sb), ("wkl", wkl_sb), ("wqh", wqh_sb),
                         ("wql", wql_sb), ("wv", wv_sb)):
            nc.sync.dma_start(out=sb,
                              in_=ins[name].rearrange("(c p) e -> p c e", p=P))
        nc.sync.dma_start(out=wo_sb, in_=wo_d)
        for nb in range(NB):
            kth_t = load_block(kth_d[:, ds(nb * 512, 512)], "kth")
            ktl_t = load_block(ktl_d[:, ds(nb * 512, 512)], "ktl")
            ps = pq_pool.tile([d, 512], f32, tag="pq")
            qk_proj(nb, kth_t, ktl_t, B, K8, wkh_sb, wkl_sb, 0, 1.0 / SQ, SK, ps)
            flush(4)
        hold_q(0)
        ps0 = pq_pool.tile([d, 512], f32, tag="pq")
        qk_proj(0, qt_blocks[0][0], qt_blocks[0][1], A, Q8, wqh_sb, wql_sb,
                1, 1.0 / SK, SQ, ps0)
        pending.extend(stats_items(0))
        hold_q(1)
        for nb in range(NB):
            vt_t = load_block(vt_d[:, ds(nb * 512, 512)], "vt", v_stream)
            v_proj(nb, vt_t)  # flushes 4 items
        for nb in range(2, NB):
            hold_q(nb)
        while pending:
            flush()

    def late_qproj(nb):
        """Projection for a deferred Q block, on the nat PSUM rotation."""
        ps3 = wo_pool.tile([P, 2, 512], f32, tag="nat", name="qproj")
        qk_proj(nb, qt_blocks[nb][0], qt_blocks[nb][1], A, Q8, wqh_sb, wql_sb,
                1, 1.0 / SK, SQ, ps3[0:d, 0, :])

    # ---- P2: eras ----
    with tc.tile_pool(name="main_ps", bufs=3, space="PSUM") as main_pool, \
         tc.tile_pool(name="av_ps", bufs=1, space="PSUM") as av_pool:

        def wo_tile(t, evict="vector"):
            """Output projection for query tile t, scaled by 1/sumexp."""
            ps3 = wo_pool.tile([P, 2, 512], f32, tag="nat", name="wo")
            ps = ps3[:, 0, :]
            nc.tensor.matmul(ps, lhsT=outT[:, ts(t, P)], rhs=wo_sb,
                             start=True, stop=True)
            o_sb = o_pool.tile([P, dim], f32, tag="o")
            if evict == "vector":
                nc.vector.tensor_scalar_mul(o_sb, ps, rinv[:, t:t + 1])
            else:
                nc.scalar.mul(o_sb, ps, rinv[:, t:t + 1])
            nc.sync.dma_start(out=out_d[ts(t, P), :], in_=o_sb)

        for c in range(NCH):
            cs = ds(c * CH, CH)
            r65 = A[:, cs]
            q8c = Q8[:, :, cs]
            seq = stats_items(c + 1) if c + 1 < NCH else []
            if c + 1 < NCH:
                late_qproj(c + 1)
            av_ps = av_pool.tile([d + 1, 512], f32, tag="av")
            att_fifo = []

            def emit_av(att_t, g, av_ps=av_ps):
                nc.tensor.matmul(av_ps, lhsT=v_sb[:, g, :], rhs=att_t,
                                 start=(g == 0), stop=(g == NT - 1))

            for g in range(NT):
                sc = main_pool.tile([P, 512], f32, tag="sc")
                nc.tensor.matmul(sc, lhsT=B[:, ts(g, P)], rhs=r65,
                                 start=True, stop=False)
                nc.tensor.matmul(sc[:, 0:256], lhsT=K8[:, :, ts(g, P)],
                                 rhs=q8c[:, :, 0:256],
                                 start=False, stop=False, perf_mode=DR)
                nc.tensor.matmul(sc[:, 256:512], lhsT=K8[:, :, ts(g, P)],
                                 rhs=q8c[:, :, 256:512],
                                 start=False, stop=True, perf_mode=DR)
                att_t = att_pool.tile([P, 512], fp16, tag="att")
                nc.scalar.activation(att_t, sc, AF.Exp)
                att_fifo.append((att_t, g))
                if len(att_fifo) > 3:
                    emit_av(*att_fifo.pop(0))
                if g % 2 == 0 and seq:
                    seq.pop(0)()
                if c > 0 and g % 8 == 6:
                    wo_tile((c - 1) * 4 + g // 8,
                            evict="vector" if g % 16 == 6 else "scalar")
            while seq:
                seq.pop(0)()
            for item in att_fifo:
                emit_av(*item)
            # evict attn_u @ v and the denominator row; gather per-tile sums
            nc.vector.tensor_copy(sumx, av_ps[d:d + 1, :])
            nc.scalar.copy(outT[:, cs], av_ps[0:d, :])
            for jj in range(4):
                t = c * 4 + jj
                nc.sync.dma_start(out=rsum[:, t:t + 1],
                                  in_=sumx[:, ds(jj * P, P)])
            nc.vector.reciprocal(rinv[:, ds(c * 4, 4)], rsum[:, ds(c * 4, 4)])

        wo_tile(NT - 4, evict="vector")
        wo_tile(NT - 3, evict="scalar")
        wo_tile(NT - 2, evict="vector")
        wo_tile(NT - 1, evict="scalar")


def _hilo(x):
    hi = np.ascontiguousarray(x.astype(np.float16))
    lo = np.ascontiguousarray((x - hi.astype(np.float32)).astype(np.float16))
    return hi, lo


def make_in_maps(Q, K, V, Wq, Wk, Wv, Wo):
    """Host-side sharding: transpose activations, split hi/lo, slice
    weights per head."""
    scale = np.float32(1.0 / np.sqrt(Wq.shape[-1]))
    QTH, QTL = _hilo(np.ascontiguousarray(np.asarray(Q, np.float32).T))
    KTH, KTL = _hilo(np.ascontiguousarray(np.asarray(K, np.float32).T))
    VT = np.ascontiguousarray(np.asarray(V, np.float32).T.astype(np.float16))
    d = Wq.shape[-1]
    in_maps = []
    for h in range(Wq.shape[0]):
        wqh, wql = _hilo(np.asarray(Wq[h], np.float32) * scale)
        wkh, wkl = _hilo(np.asarray(Wk[h], np.float32))
        in_maps.append({
            "QTH": QTH, "QTL": QTL, "KTH": KTH, "KTL": KTL, "VT": VT,
            "wqh": wqh, "wql": wql, "wkh": wkh, "wkl": wkl,
            "wv": np.ascontiguousarray(np.asarray(Wv[h], np.float32)
                                       .astype(np.float16)),
            "wo": np.ascontiguousarray(np.asarray(Wo[h * d:(h + 1) * d, :],
                                                  np.float32)),
        })
    return in_maps


_CACHE = {}


def _build_and_compile(n=N, dim=DIM, d=D, num_cores=H, repeats=1):
    import concourse.bass as bass
    import concourse.mybir as mybir
    import concourse.tile as tile
    from concourse import bacc

    key = (n, dim, d, num_cores, repeats)
    if key in _CACHE:
        return _CACHE[key]
    nc = bacc.Bacc("TRN2", target_bir_lowering=False, debug=False,
                   num_devices=num_cores)
    f32 = mybir.dt.float32
    f32r = mybir.dt.float32r
    fp16 = mybir.dt.float16
    ins = {}
    for name in ("QTH", "QTL", "KTH", "KTL", "VT"):
        ins[name] = nc.dram_tensor(name, [dim, n], fp16, kind="ExternalInput").ap()
    for name in ("wqh", "wql", "wkh", "wkl", "wv"):
        ins[name] = nc.dram_tensor(name, [dim, d], fp16, kind="ExternalInput").ap()
    ins["wo"] = nc.dram_tensor("wo", [d, dim], f32r, kind="ExternalInput").ap()
    outs = {"out": nc.dram_tensor("out", [n, dim], f32, kind="ExternalOutput").ap()}
    with tile.TileContext(nc) as tc:
        for _rep in range(repeats):
            with ExitStack() as ctx:
                build_head_kernel(ctx, tc, outs, ins, n=n, dim=dim, d=d)
    nc.compile()
    _CACHE[key] = nc
    return nc


def run_on_hw(in_maps, trace=False, **kwargs):
    from concourse.bass_utils import run_bass_kernel_spmd

    nc = _build_and_compile(num_cores=len(in_maps))
    return run_bass_kernel_spmd(nc, in_maps, core_ids=list(range(len(in_maps))),
                                trace=trace, **kwargs)


def kernel(Q, K, V, Wq, Wk, Wv, Wo):
    in_maps = make_in_maps(np.asarray(Q), np.asarray(K), np.asarray(V),
                           np.asarray(Wq), np.asarray(Wk), np.asarray(Wv),
                           np.asarray(Wo))
    res = run_on_hw(in_maps)
    out = np.zeros((N, DIM), dtype=np.float64)
    for r in res.results:
        out += r["out"].astype(np.float64)
    return out.astype(np.float32)


if __name__ == "__main__":
    from concourse.timeline_sim import TimelineSim
    nc = _build_and_compile(num_cores=1)
    print("makespan:", TimelineSim(nc).simulate())


# revision 5
# speedup vs baseline: 1.0474x; 1.0474x over previous
"""Multi-head attention on 8 Trainium2 NeuronCores (head-parallel), v3.

Problem: Q,K,V [4096,512] fp32; Wq/Wk/Wv [8,512,64]; Wo [512,512].
  out = concat_h(softmax(QWq_h (KWk_h)^T / sqrt(64)) VWv_h) @ Wo

Sharding: one head per core; each core also computes its slice of the
output projection (out_h @ Wo[64h:64h+64, :]); the host sums the 8
partial [4096,512] outputs.

Numerics (float32r matmuls only carry ~14 mantissa bits on this target,
so the near-fp32 paths are built from fp16/fp8 pieces):
  proj   q,k in 3-term fp16 hi/lo (Q = Qh+Ql, W = Wh+Wl, drop lolo,
         ~22-bit); v single-pass fp16.
  stats  per-row score max from the fp16 hi parts (shift-invariance of
         softmax cancels the ~+-3 estimate error); DVE
         tensor_tensor_reduce folds two 512-key chunks per op.
  main   transposed scores = fp16 hi.hi matmul with a 65th contraction
         row (k side -1, q side rowmax) so PSUM gets s - rowmax, plus
         fp8e4m3 DoubleRow matmuls carrying both cross terms
         (lo operands pre-scaled by 2^8/2^7 into e4m3 range).
  att.V  fp16 attn (est window ~e^4) x fp16 v; a ones column in v emits
         the softmax denominator row.
  wo     f32r out-projection (final linear layer tolerates 14-bit),
         scaled by 1/denominator on eviction.
"""

from contextlib import ExitStack

import numpy as np

N = 4096
DIM = 512
H = 8
D = 64
P = 128
CH = 512   # queries per era
NCH = N // CH
NT = N // P      # 32 key tiles
KC = DIM // P    # 4 contraction chunks for projections
NB = N // 512    # 8 input column blocks
SK = 256.0       # fp8 scaling, slot0: (k_lo*SK) . (q_hi/SK)
SQ = 128.0       # fp8 scaling, slot1: (k_hi/SQ) . (q_lo*SQ)


def build_head_kernel(ctx, tc, outs, ins, n=N, dim=DIM, d=D):
    import concourse.bass as bass
    import concourse.mybir as mybir
    from concourse.bass import ts, ds

    nc = tc.nc
    f32 = mybir.dt.float32
    f32r = mybir.dt.float32r
    fp16 = mybir.dt.float16
    f8e4 = mybir.dt.float8e4
    AF = mybir.ActivationFunctionType
    AX = mybir.AxisListType
    DR = mybir.MatmulPerfMode.DoubleRow

    qth_d, qtl_d = ins["QTH"], ins["QTL"]
    kth_d, ktl_d = ins["KTH"], ins["KTL"]
    vt_d = ins["VT"]
    wo_d = ins["wo"]
    out_d = outs["out"]

    singles = ctx.enter_context(tc.tile_pool(name="singles", bufs=1))

    A = singles.tile([d + 1, n], fp16)     # q16; row d = rowmax est (fp16)
    B = singles.tile([d + 1, n], fp16)     # k16; row d = -1
    Q8 = singles.tile([d, 2, n], f8e4)     # slots: (q16l*SQ, q16/SK)
    K8 = singles.tile([d, 2, n], f8e4)     # slots: (k16/SQ, k16l*SK)
    v_sb = singles.tile([P, NT, d + 1], fp16)  # v tiles + ones column
    outT = singles.tile([d, n], f32r)      # attn_u @ v (unnormalized)
    sumx = singles.tile([1, CH], f32)      # denominators (per era)
    rsum = singles.tile([P, NT], f32)
    rinv = singles.tile([P, NT], f32)
    wqh_sb = singles.tile([P, KC, d], fp16)
    wql_sb = singles.tile([P, KC, d], fp16)
    wkh_sb = singles.tile([P, KC, d], fp16)
    wkl_sb = singles.tile([P, KC, d], fp16)
    wv_sb = singles.tile([P, KC, d], fp16)
    wo_sb = singles.tile([d, dim], f32r)

    # PSUM pools: nat 2x[128,2x512] = 4 banks (shared with wo/qproj),
    # main 3x[128,512] = 3, av 1.
    nat_pool = ctx.enter_context(tc.tile_pool(name="nat_ps", bufs=2, space="PSUM"))
    wo_pool = nat_pool
    att_pool = ctx.enter_context(tc.tile_pool(name="att", bufs=6))
    nmax_pool = ctx.enter_context(tc.tile_pool(name="nmax", bufs=5))
    o_pool = ctx.enter_context(tc.tile_pool(name="osb", bufs=4))
    qt_hold = ctx.enter_context(tc.tile_pool(name="qt_hold", bufs=4))

    nmax_tiles = {}

    def nat_item(c, jt, j):
        """Stats for era-c q-tile jt: key chunk j (1024 keys as two 512
        halves); one tensor_tensor_reduce folds both halves."""
        qs = ts(c * 4 + jt, P)
        st = nat_pool.tile([P, 2, 512], f32, tag="nat")
        nc.tensor.matmul(st[:, 0, :], lhsT=A[0:d, qs], rhs=B[0:d, ts(2 * j, 512)],
                         start=True, stop=True)
        nc.tensor.matmul(st[:, 1, :], lhsT=A[0:d, qs], rhs=B[0:d, ts(2 * j + 1, 512)],
                         start=True, stop=True)
        if j == 0:
            nmax_tiles[jt] = nmax_pool.tile([P, 5], fp16, tag="nmax", name="nmax")
        nm = nmax_tiles[jt]
        nc.vector.reduce_max(nm[:, j:j + 1], st, axis=AX.XY)
        if j == 3:
            nc.vector.reduce_max(nm[:, 4:5], nm[:, 0:4], axis=AX.X)
            nc.sync.dma_start(out=A[d:d + 1, ds(c * CH + jt * P, P)],
                              in_=nm[:, 4:5])

    def stats_items(c):
        """16 stats items for era c (4 q-tiles x 4 key chunks)."""
        return [lambda c=c, jt=jt, j=j: nat_item(c, jt, j)
                for j in range(4) for jt in range(4)]

    # ---- P1: load + projections, era-0 stats folded in ----
    pending = []

    def flush(k=1):
        for _ in range(k):
            if pending:
                pending.pop(0)()

    qt_blocks = {}

    def hold_q(nb):
        th = qt_hold.tile([P, KC, 512], fp16, tag="qtbh", name=f"qtbh{nb}")
        nc.sync.dma_start(out=th, in_=qth_d[:, ds(nb * 512, 512)]
                          .rearrange("(c p) x -> p c x", p=P))
        tl = qt_hold.tile([P, KC, 512], fp16, tag="qtbl", name=f"qtbl{nb}")
        nc.sync.dma_start(out=tl, in_=qtl_d[:, ds(nb * 512, 512)]
                          .rearrange("(c p) x -> p c x", p=P))
        qt_blocks[nb] = (th, tl)

    def qk_proj(nb, src_h, src_l, dst, dst8, wh, wl, hi_slot, s_hi, s_lo, ps):
        """3-term fp16 projection + fp16/fp8 operand prep.  dst rows 0:d
        get the fp16 hi; dst8 slot hi_slot = hi*s_hi, the other = lo*s_lo,
        so the DoubleRow matmul pairs k_hi.q_lo and k_lo.q_hi."""
        nbs = ds(nb * 512, 512)
        terms = [(wh, src_h), (wh, src_l), (wl, src_h)]
        for i, (w, x) in enumerate(terms):
            for kc in range(KC):
                nc.tensor.matmul(ps, lhsT=w[:, kc, :], rhs=x[:, kc, :],
                                 start=(i == 0 and kc == 0),
                                 stop=(i == 2 and kc == KC - 1))
        nc.scalar.copy(dst[0:d, nbs], ps)
        lo = o_pool.tile([d, 512], fp16, tag="lo", name="lo")
        nc.vector.tensor_sub(lo, ps, dst[0:d, nbs])
        nc.scalar.mul(dst8[:, hi_slot, nbs], dst[0:d, nbs], float(s_hi))
        nc.vector.tensor_scalar_mul(dst8[:, 1 - hi_slot, nbs], lo, float(s_lo))

    with tc.tile_pool(name="qk_stream", bufs=3) as qk_stream, \
         tc.tile_pool(name="v_stream", bufs=2) as v_stream, \
         tc.tile_pool(name="pq_ps", bufs=1, space="PSUM") as pq_pool, \
         tc.tile_pool(name="pv_ps", bufs=2, space="PSUM") as pv_pool:

        def load_block(t_d, tag, pool=None):
            pool = pool or qk_stream
            t = pool.tile([P, KC, 512], fp16, tag=tag, name=tag)
            nc.sync.dma_start(out=t, in_=t_d.rearrange("(c p) x -> p c x", p=P))
            return t

        def v_proj(nb, vt_t):
            ps = pv_pool.tile([P, 4, d], f32, tag="pv")
            for i in range(4):
                for kc in range(KC):
                    nc.tensor.matmul(ps[:, i, :], lhsT=vt_t[:, kc, ts(i, P)],
                                     rhs=wv_sb[:, kc, :],
                                     start=(kc == 0), stop=(kc == KC - 1))
                flush()
            nc.vector.tensor_copy(v_sb[:, ds(4 * nb, 4), 0:d], ps)

        # DMA order: weights, K0-7 (hi+lo), Q0, V0-7, Q1-7
        nc.vector.memset(B[d:d + 1, :], -1.0)
        nc.vector.memset(v_sb[:, :, d:d + 1], 1.0)
        for name, sb in (("wkh", wkh_sb), ("wkl", wkl_sb), ("wqh", wqh_sb),
                         ("wql", wql_sb), ("wv", wv_sb)):
            nc.sync.dma_start(out=sb,
                              in_=ins[name].rearrange("(c p) e -> p c e", p=P))
        nc.sync.dma_start(out=wo_sb, in_=wo_d)
        for nb in range(NB):
            kth_t = load_block(kth_d[:, ds(nb * 512, 512)], "kth")
            ktl_t = load_block(ktl_d[:, ds(nb * 512, 512)], "ktl")
            ps = pq_pool.tile([d, 512], f32, tag="pq")
            qk_proj(nb, kth_t, ktl_t, B, K8, wkh_sb, wkl_sb, 0, 1.0 / SQ, SK, ps)
            if nb == 1:
                hold_q(0)
                ps0 = pq_pool.tile([d, 512], f32, tag="pq")
                qk_proj(0, qt_blocks[0][0], qt_blocks[0][1], A, Q8,
                        wqh_sb, wql_sb, 1, 1.0 / SK, SQ, ps0)
                pending.extend(stats_items(0))
            if nb in (2, 3, 5, 7):
                flush(4)
        hold_q(1)
        for nb in range(NB):
            vt_t = load_block(vt_d[:, ds(nb * 512, 512)], "vt", v_stream)
            v_proj(nb, vt_t)  # flushes 4 items
        for nb in range(2, NB):
            hold_q(nb)
        while pending:
            flush()

    def late_qproj(nb):
        """Projection for a deferred Q block, on the nat PSUM rotation."""
        ps3 = wo_pool.tile([P, 2, 512], f32, tag="nat", name="qproj")
        qk_proj(nb, qt_blocks[nb][0], qt_blocks[nb][1], A, Q8, wqh_sb, wql_sb,
                1, 1.0 / SK, SQ, ps3[0:d, 0, :])

    # ---- P2: eras ----
    with tc.tile_pool(name="main_ps", bufs=3, space="PSUM") as main_pool, \
         tc.tile_pool(name="av_ps", bufs=1, space="PSUM") as av_pool:

        def wo_tile(t, evict="vector"):
            """Output projection for query tile t, scaled by 1/sumexp."""
            ps3 = wo_pool.tile([P, 2, 512], f32, tag="nat", name="wo")
            ps = ps3[:, 0, :]
            nc.tensor.matmul(ps, lhsT=outT[:, ts(t, P)], rhs=wo_sb,
                             start=True, stop=True)
            o_sb = o_pool.tile([P, dim], f32, tag="o")
            if evict == "vector":
                nc.vector.tensor_scalar_mul(o_sb, ps, rinv[:, t:t + 1])
            else:
                nc.scalar.mul(o_sb, ps, rinv[:, t:t + 1])
            nc.sync.dma_start(out=out_d[ts(t, P), :], in_=o_sb)

        for c in range(NCH):
            cs = ds(c * CH, CH)
            r65 = A[:, cs]
            q8c = Q8[:, :, cs]
            seq = stats_items(c + 1) if c + 1 < NCH else []
            if c + 1 < NCH:
                late_qproj(c + 1)
            av_ps = av_pool.tile([d + 1, 512], f32, tag="av")
            att_fifo = []

            def emit_av(att_t, g, av_ps=av_ps):
                nc.tensor.matmul(av_ps, lhsT=v_sb[:, g, :], rhs=att_t,
                                 start=(g == 0), stop=(g == NT - 1))

            for g in range(NT):
                sc = main_pool.tile([P, 512], f32, tag="sc")
                nc.tensor.matmul(sc, lhsT=B[:, ts(g, P)], rhs=r65,
                                 start=True, stop=False)
                nc.tensor.matmul(sc[:, 0:256], lhsT=K8[:, :, ts(g, P)],
                                 rhs=q8c[:, :, 0:256],
                                 start=False, stop=False, perf_mode=DR)
                nc.tensor.matmul(sc[:, 256:512], lhsT=K8[:, :, ts(g, P)],
                                 rhs=q8c[:, :, 256:512],
                                 start=False, stop=True, perf_mode=DR)
                att_t = att_pool.tile([P, 512], fp16, tag="att")
                nc.scalar.activation(att_t, sc, AF.Exp)
                att_fifo.append((att_t, g))
                if len(att_fifo) > 3:
                    emit_av(*att_fifo.pop(0))
                if g % 2 == 0 and seq:
                    seq.pop(0)()
                if c > 0 and g % 8 == 6:
                    wo_tile((c - 1) * 4 + g // 8,
                            evict="vector" if g % 16 == 6 else "scalar")
            while seq:
                seq.pop(0)()
            for item in att_fifo:
                emit_av(*item)
            # evict attn_u @ v and the denominator row; gather per-tile sums
            nc.vector.tensor_copy(sumx, av_ps[d:d + 1, :])
            nc.scalar.copy(outT[:, cs], av_ps[0:d, :])
            for jj in range(4):
                t = c * 4 + jj
                nc.sync.dma_start(out=rsum[:, t:t + 1],
                                  in_=sumx[:, ds(jj * P, P)])
            nc.vector.reciprocal(rinv[:, ds(c * 4, 4)], rsum[:, ds(c * 4, 4)])

        wo_tile(NT - 4, evict="vector")
        wo_tile(NT - 3, evict="scalar")
        wo_tile(NT - 2, evict="vector")
        wo_tile(NT - 1, evict="scalar")


def _hilo(x):
    hi = np.ascontiguousarray(x.astype(np.float16))
    lo = np.ascontiguousarray((x - hi.astype(np.float32)).astype(np.float16))
    return hi, lo


def make_in_maps(Q, K, V, Wq, Wk, Wv, Wo):
    """Host-side sharding: transpose activations, split hi/lo, slice
    weights per head."""
    scale = np.float32(1.0 / np.sqrt(Wq.shape[-1]))
    QTH, QTL = _hilo(np.ascontiguousarray(np.asarray(Q, np.float32).T))
    KTH, KTL = _hilo(np.ascontiguousarray(np.asarray(K, np.float32).T))
    VT = np.ascontiguousarray(np.asarray(V, np.float32).T.astype(np.float16))
    d = Wq.shape[-1]
    in_maps = []
    for h in range(Wq.shape[0]):
        wqh, wql = _hilo(np.asarray(Wq[h], np.float32) * scale)
        wkh, wkl = _hilo(np.asarray(Wk[h], np.float32))
        in_maps.append({
            "QTH": QTH, "QTL": QTL, "KTH": KTH, "KTL": KTL, "VT": VT,
            "wqh": wqh, "wql": wql, "wkh": wkh, "wkl": wkl,
            "wv": np.ascontiguousarray(np.asarray(Wv[h], np.float32)
                                       .astype(np.float16)),
            "wo": np.ascontiguousarray(np.asarray(Wo[h * d:(h + 1) * d, :],
                                                  np.float32)),
        })
    return in_maps


_CACHE = {}


def _build_and_compile(n=N, dim=DIM, d=D, num_cores=H, repeats=1):
    import concourse.bass as bass
    import concourse.mybir as mybir
    import concourse.tile as tile
    from concourse import bacc

    key = (n, dim, d, num_cores, repeats)
    if key in _CACHE:
        return _CACHE[key]
    nc = bacc.Bacc("TRN2", target_bir_lowering=False, debug=False,
                   num_devices=num_cores)
    f32 = mybir.dt.float32
    f32r = mybir.dt.float32r
    fp16 = mybir.dt.float16
    ins = {}
    for name in ("QTH", "QTL", "KTH", "KTL", "VT"):
        ins[name] = nc.dram_tensor(name, [dim, n], fp16, kind="ExternalInput").ap()
    for name in ("wqh", "wql", "wkh", "wkl", "wv"):
        ins[name] = nc.dram_tensor(name, [dim, d], fp16, kind="ExternalInput").ap()
    ins["wo"] = nc.dram_tensor("wo", [d, dim], f32r, kind="ExternalInput").ap()
    outs = {"out": nc.dram_tensor("out", [n, dim], f32, kind="ExternalOutput").ap()}
    with tile.TileContext(nc) as tc:
        for _rep in range(repeats):
            with ExitStack() as ctx:
                build_head_kernel(ctx, tc, outs, ins, n=n, dim=dim, d=d)
    nc.compile()
    _CACHE[key] = nc
    return nc


def run_on_hw(in_maps, trace=False, **kwargs):
    from concourse.bass_utils import run_bass_kernel_spmd

    nc = _build_and_compile(num_cores=len(in_maps))
    return run_bass_kernel_spmd(nc, in_maps, core_ids=list(range(len(in_maps))),
                                trace=trace, **kwargs)


def kernel(Q, K, V, Wq, Wk, Wv, Wo):
    in_maps = make_in_maps(np.asarray(Q), np.asarray(K), np.asarray(V),
                           np.asarray(Wq), np.asarray(Wk), np.asarray(Wv),
                           np.asarray(Wo))
    res = run_on_hw(in_maps)
    out = np.zeros((N, DIM), dtype=np.float64)
    for r in res.results:
        out += r["out"].astype(np.float64)
    return out.astype(np.float32)


if __name__ == "__main__":
    from concourse.timeline_sim import TimelineSim
    nc = _build_and_compile(num_cores=1)
    print("makespan:", TimelineSim(nc).simulate())


# revision 6
# speedup vs baseline: 1.0475x; 1.0001x over previous
"""Multi-head attention on 8 Trainium2 NeuronCores (head-parallel), v3.

Problem: Q,K,V [4096,512] fp32; Wq/Wk/Wv [8,512,64]; Wo [512,512].
  out = concat_h(softmax(QWq_h (KWk_h)^T / sqrt(64)) VWv_h) @ Wo

Sharding: one head per core; each core also computes its slice of the
output projection (out_h @ Wo[64h:64h+64, :]); the host sums the 8
partial [4096,512] outputs.

Numerics (float32r matmuls only carry ~14 mantissa bits on this target,
so the near-fp32 paths are built from fp16/fp8 pieces):
  proj   q,k in 3-term fp16 hi/lo (Q = Qh+Ql, W = Wh+Wl, drop lolo,
         ~22-bit); v single-pass fp16.
  stats  per-row score max from the fp16 hi parts (shift-invariance of
         softmax cancels the ~+-3 estimate error); DVE
         tensor_tensor_reduce folds two 512-key chunks per op.
  main   transposed scores = fp16 hi.hi matmul with a 65th contraction
         row (k side -1, q side rowmax) so PSUM gets s - rowmax, plus
         fp8e4m3 DoubleRow matmuls carrying both cross terms
         (lo operands pre-scaled by 2^8/2^7 into e4m3 range).
  att.V  fp16 attn (est window ~e^4) x fp16 v; a ones column in v emits
         the softmax denominator row.
  wo     f32r out-projection (final linear layer tolerates 14-bit),
         scaled by 1/denominator on eviction.
"""

from contextlib import ExitStack

import numpy as np

N = 4096
DIM = 512
H = 8
D = 64
P = 128
CH = 512   # queries per era
NCH = N // CH
NT = N // P      # 32 key tiles
KC = DIM // P    # 4 contraction chunks for projections
NB = N // 512    # 8 input column blocks
SK = 256.0       # fp8 scaling, slot0: (k_lo*SK) . (q_hi/SK)
SQ = 128.0       # fp8 scaling, slot1: (k_hi/SQ) . (q_lo*SQ)


def build_head_kernel(ctx, tc, outs, ins, n=N, dim=DIM, d=D):
    import concourse.bass as bass
    import concourse.mybir as mybir
    from concourse.bass import ts, ds

    nc = tc.nc
    f32 = mybir.dt.float32
    f32r = mybir.dt.float32r
    fp16 = mybir.dt.float16
    f8e4 = mybir.dt.float8e4
    AF = mybir.ActivationFunctionType
    AX = mybir.AxisListType
    DR = mybir.MatmulPerfMode.DoubleRow

    qth_d, qtl_d = ins["QTH"], ins["QTL"]
    kth_d, ktl_d = ins["KTH"], ins["KTL"]
    vt_d = ins["VT"]
    wo_d = ins["wo"]
    out_d = outs["out"]

    singles = ctx.enter_context(tc.tile_pool(name="singles", bufs=1))

    A = singles.tile([d + 1, n], fp16)     # q16; row d = rowmax est (fp16)
    B = singles.tile([d + 1, n], fp16)     # k16; row d = -1
    Q8 = singles.tile([d, 2, n], f8e4)     # slots: (q16l*SQ, q16/SK)
    K8 = singles.tile([d, 2, n], f8e4)     # slots: (k16/SQ, k16l*SK)
    v_sb = singles.tile([P, NT, d + 1], fp16)  # v tiles + ones column
    outT = singles.tile([d, n], f32r)      # attn_u @ v (unnormalized)
    sumx = singles.tile([1, CH], f32)      # denominators (per era)
    rsum = singles.tile([P, NT], f32)
    rinv = singles.tile([P, NT], f32)
    wqh_sb = singles.tile([P, KC, d], fp16)
    wql_sb = singles.tile([P, KC, d], fp16)
    wkh_sb = singles.tile([P, KC, d], fp16)
    wkl_sb = singles.tile([P, KC, d], fp16)
    wv_sb = singles.tile([P, KC, d], fp16)
    wo_sb = singles.tile([d, dim], f32r)

    # PSUM pools: nat 2x[128,2x512] = 4 banks (shared with wo/qproj),
    # main 3x[128,512] = 3, av 1.
    nat_pool = ctx.enter_context(tc.tile_pool(name="nat_ps", bufs=2, space="PSUM"))
    wo_pool = nat_pool
    att_pool = ctx.enter_context(tc.tile_pool(name="att", bufs=6))
    nmax_pool = ctx.enter_context(tc.tile_pool(name="nmax", bufs=5))
    o_pool = ctx.enter_context(tc.tile_pool(name="osb", bufs=4))
    qt_hold = ctx.enter_context(tc.tile_pool(name="qt_hold", bufs=4))

    nmax_tiles = {}

    def nat_item(c, jt, j):
        """Stats for era-c q-tile jt: key chunk j (1024 keys as two 512
        halves); one tensor_tensor_reduce folds both halves."""
        qs = ts(c * 4 + jt, P)
        st = nat_pool.tile([P, 2, 512], f32, tag="nat")
        nc.tensor.matmul(st[:, 0, :], lhsT=A[0:d, qs], rhs=B[0:d, ts(2 * j, 512)],
                         start=True, stop=True)
        nc.tensor.matmul(st[:, 1, :], lhsT=A[0:d, qs], rhs=B[0:d, ts(2 * j + 1, 512)],
                         start=True, stop=True)
        if j == 0:
            nmax_tiles[jt] = nmax_pool.tile([P, 5], fp16, tag="nmax", name="nmax")
        nm = nmax_tiles[jt]
        nc.vector.reduce_max(nm[:, j:j + 1], st, axis=AX.XY)
        if j == 3:
            nc.vector.reduce_max(nm[:, 4:5], nm[:, 0:4], axis=AX.X)
            nc.sync.dma_start(out=A[d:d + 1, ds(c * CH + jt * P, P)],
                              in_=nm[:, 4:5])

    def stats_items(c):
        """16 stats items for era c (4 q-tiles x 4 key chunks)."""
        return [lambda c=c, jt=jt, j=j: nat_item(c, jt, j)
                for j in range(4) for jt in range(4)]

    # ---- P1: load + projections, era-0 stats folded in ----
    pending = []

    def flush(k=1):
        for _ in range(k):
            if pending:
                pending.pop(0)()

    qt_blocks = {}

    def hold_q(nb):
        th = qt_hold.tile([P, KC, 512], fp16, tag="qtbh", name=f"qtbh{nb}")
        nc.sync.dma_start(out=th, in_=qth_d[:, ds(nb * 512, 512)]
                          .rearrange("(c p) x -> p c x", p=P))
        tl = qt_hold.tile([P, KC, 512], fp16, tag="qtbl", name=f"qtbl{nb}")
        nc.sync.dma_start(out=tl, in_=qtl_d[:, ds(nb * 512, 512)]
                          .rearrange("(c p) x -> p c x", p=P))
        qt_blocks[nb] = (th, tl)

    def qk_proj(nb, src_h, src_l, dst, dst8, wh, wl, hi_slot, s_hi, s_lo, ps):
        """3-term fp16 projection + fp16/fp8 operand prep.  dst rows 0:d
        get the fp16 hi; dst8 slot hi_slot = hi*s_hi, the other = lo*s_lo,
        so the DoubleRow matmul pairs k_hi.q_lo and k_lo.q_hi."""
        nbs = ds(nb * 512, 512)
        terms = [(wh, src_h), (wh, src_l), (wl, src_h)]
        for i, (w, x) in enumerate(terms):
            for kc in range(KC):
                nc.tensor.matmul(ps, lhsT=w[:, kc, :], rhs=x[:, kc, :],
                                 start=(i == 0 and kc == 0),
                                 stop=(i == 2 and kc == KC - 1))
        nc.scalar.copy(dst[0:d, nbs], ps)
        lo = o_pool.tile([d, 512], fp16, tag="lo", name="lo")
        nc.vector.tensor_sub(lo, ps, dst[0:d, nbs])
        nc.scalar.mul(dst8[:, hi_slot, nbs], dst[0:d, nbs], float(s_hi))
        nc.vector.tensor_scalar_mul(dst8[:, 1 - hi_slot, nbs], lo, float(s_lo))

    with tc.tile_pool(name="qk_stream", bufs=3) as qk_stream, \
         tc.tile_pool(name="v_stream", bufs=2) as v_stream, \
         tc.tile_pool(name="pq_ps", bufs=1, space="PSUM") as pq_pool, \
         tc.tile_pool(name="pv_ps", bufs=2, space="PSUM") as pv_pool:

        def load_block(t_d, tag, pool=None):
            pool = pool or qk_stream
            t = pool.tile([P, KC, 512], fp16, tag=tag, name=tag)
            nc.sync.dma_start(out=t, in_=t_d.rearrange("(c p) x -> p c x", p=P))
            return t

        def v_proj(nb, vt_t):
            ps = pv_pool.tile([P, 4, d], f32, tag="pv")
            for i in range(4):
                for kc in range(KC):
                    nc.tensor.matmul(ps[:, i, :], lhsT=vt_t[:, kc, ts(i, P)],
                                     rhs=wv_sb[:, kc, :],
                                     start=(kc == 0), stop=(kc == KC - 1))
                flush()
            nc.vector.tensor_copy(v_sb[:, ds(4 * nb, 4), 0:d], ps)

        # DMA order: weights, K0-7 (hi+lo), Q0, V0-7, Q1-7
        nc.vector.memset(B[d:d + 1, :], -1.0)
        nc.vector.memset(v_sb[:, :, d:d + 1], 1.0)
        for name, sb in (("wkh", wkh_sb), ("wkl", wkl_sb), ("wqh", wqh_sb),
                         ("wql", wql_sb), ("wv", wv_sb)):
            nc.sync.dma_start(out=sb,
                              in_=ins[name].rearrange("(c p) e -> p c e", p=P))
        nc.sync.dma_start(out=wo_sb, in_=wo_d)
        for nb in range(NB):
            kth_t = load_block(kth_d[:, ds(nb * 512, 512)], "kth")
            ktl_t = load_block(ktl_d[:, ds(nb * 512, 512)], "ktl")
            ps = pq_pool.tile([d, 512], f32, tag="pq")
            qk_proj(nb, kth_t, ktl_t, B, K8, wkh_sb, wkl_sb, 0, 1.0 / SQ, SK, ps)
            if nb == 1:
                hold_q(0)
                ps0 = pq_pool.tile([d, 512], f32, tag="pq")
                qk_proj(0, qt_blocks[0][0], qt_blocks[0][1], A, Q8,
                        wqh_sb, wql_sb, 1, 1.0 / SK, SQ, ps0)
                pending.extend(stats_items(0))
            if nb in (2, 3, 5, 7):
                flush(4)
        hold_q(1)
        for nb in range(NB):
            vt_t = load_block(vt_d[:, ds(nb * 512, 512)], "vt", v_stream)
            v_proj(nb, vt_t)  # flushes 4 items
        for nb in range(2, NB):
            hold_q(nb)
        while pending:
            flush()

    def late_qproj(nb):
        """Projection for a deferred Q block, on the nat PSUM rotation."""
        ps3 = wo_pool.tile([P, 2, 512], f32, tag="nat", name="qproj")
        qk_proj(nb, qt_blocks[nb][0], qt_blocks[nb][1], A, Q8, wqh_sb, wql_sb,
                1, 1.0 / SK, SQ, ps3[0:d, 0, :])

    # ---- P2: eras ----
    with tc.tile_pool(name="main_ps", bufs=3, space="PSUM") as main_pool, \
         tc.tile_pool(name="av_ps", bufs=1, space="PSUM") as av_pool:

        def wo_tile(t, evict="vector"):
            """Output projection for query tile t, scaled by 1/sumexp."""
            ps3 = wo_pool.tile([P, 2, 512], f32, tag="nat", name="wo")
            ps = ps3[:, 0, :]
            nc.tensor.matmul(ps, lhsT=outT[:, ts(t, P)], rhs=wo_sb,
                             start=True, stop=True)
            o_sb = o_pool.tile([P, dim], f32, tag="o")
            if evict == "vector":
                nc.vector.tensor_scalar_mul(o_sb, ps, rinv[:, t:t + 1])
            else:
                nc.scalar.mul(o_sb, ps, rinv[:, t:t + 1])
            nc.sync.dma_start(out=out_d[ts(t, P), :], in_=o_sb)

        for c in range(NCH):
            cs = ds(c * CH, CH)
            r65 = A[:, cs]
            q8c = Q8[:, :, cs]
            seq = stats_items(c + 1) if c + 1 < NCH else []
            if c + 1 < NCH:
                late_qproj(c + 1)
            av_ps = av_pool.tile([d + 1, 512], f32, tag="av")
            att_fifo = []

            def emit_av(att_t, g, av_ps=av_ps):
                nc.tensor.matmul(av_ps, lhsT=v_sb[:, g, :], rhs=att_t,
                                 start=(g == 0), stop=(g == NT - 1))

            for g in range(NT):
                sc = main_pool.tile([P, 512], f32, tag="sc")
                nc.tensor.matmul(sc, lhsT=B[:, ts(g, P)], rhs=r65,
                                 start=True, stop=False)
                nc.tensor.matmul(sc[:, 0:256], lhsT=K8[:, :, ts(g, P)],
                                 rhs=q8c[:, :, 0:256],
                                 start=False, stop=False, perf_mode=DR)
                nc.tensor.matmul(sc[:, 256:512], lhsT=K8[:, :, ts(g, P)],
                                 rhs=q8c[:, :, 256:512],
                                 start=False, stop=True, perf_mode=DR)
                att_t = att_pool.tile([P, 512], fp16, tag="att")
                nc.scalar.activation(att_t, sc, AF.Exp)
                att_fifo.append((att_t, g))
                if len(att_fifo) > 3:
                    emit_av(*att_fifo.pop(0))
                if seq and (g % 2 == 0 or g >= 16):
                    seq.pop(0)()
                if c > 0 and g % 8 == 6:
                    wo_tile((c - 1) * 4 + g // 8,
                            evict="vector" if g % 16 == 6 else "scalar")
            while seq:
                seq.pop(0)()
            for item in att_fifo:
                emit_av(*item)
            # evict attn_u @ v and the denominator row; gather per-tile sums
            nc.vector.tensor_copy(sumx, av_ps[d:d + 1, :])
            nc.scalar.copy(outT[:, cs], av_ps[0:d, :])
            for jj in range(4):
                t = c * 4 + jj
                nc.sync.dma_start(out=rsum[:, t:t + 1],
                                  in_=sumx[:, ds(jj * P, P)])
            nc.vector.reciprocal(rinv[:, ds(c * 4, 4)], rsum[:, ds(c * 4, 4)])

        wo_tile(NT - 4, evict="vector")
        wo_tile(NT - 3, evict="scalar")
        wo_tile(NT - 2, evict="vector")
        wo_tile(NT - 1, evict="scalar")


def _hilo(x):
    hi = np.ascontiguousarray(x.astype(np.float16))
    lo = np.ascontiguousarray((x - hi.astype(np.float32)).astype(np.float16))
    return hi, lo


def make_in_maps(Q, K, V, Wq, Wk, Wv, Wo):
    """Host-side sharding: transpose activations, split hi/lo, slice
    weights per head."""
    scale = np.float32(1.0 / np.sqrt(Wq.shape[-1]))
    QTH, QTL = _hilo(np.ascontiguousarray(np.asarray(Q, np.float32).T))
    KTH, KTL = _hilo(np.ascontiguousarray(np.asarray(K, np.float32).T))
    VT = np.ascontiguousarray(np.asarray(V, np.float32).T.astype(np.float16))
    d = Wq.shape[-1]
    in_maps = []
    for h in range(Wq.shape[0]):
        wqh, wql = _hilo(np.asarray(Wq[h], np.float32) * scale)
        wkh, wkl = _hilo(np.asarray(Wk[h], np.float32))
        in_maps.append({
            "QTH": QTH, "QTL": QTL, "KTH": KTH, "KTL": KTL, "VT": VT,
            "wqh": wqh, "wql": wql, "wkh": wkh, "wkl": wkl,
            "wv": np.ascontiguousarray(np.asarray(Wv[h], np.float32)
                                       .astype(np.float16)),
            "wo": np.ascontiguousarray(np.asarray(Wo[h * d:(h + 1) * d, :],
                                                  np.float32)),
        })
    return in_maps


_CACHE = {}


def _build_and_compile(n=N, dim=DIM, d=D, num_cores=H, repeats=1):
    import concourse.bass as bass
    import concourse.mybir as mybir
    import concourse.tile as tile
    from concourse import bacc

    key = (n, dim, d, num_cores, repeats)
    if key in _CACHE:
        return _CACHE[key]
    nc = bacc.Bacc("TRN2", target_bir_lowering=False, debug=False,
                   num_devices=num_cores)
    f32 = mybir.dt.float32
    f32r = mybir.dt.float32r
    fp16 = mybir.dt.float16
    ins = {}
    for name in ("QTH", "QTL", "KTH", "KTL", "VT"):
        ins[name] = nc.dram_tensor(name, [dim, n], fp16, kind="ExternalInput").ap()
    for name in ("wqh", "wql", "wkh", "wkl", "wv"):
        ins[name] = nc.dram_tensor(name, [dim, d], fp16, kind="ExternalInput").ap()
    ins["wo"] = nc.dram_tensor("wo", [d, dim], f32r, kind="ExternalInput").ap()
    outs = {"out": nc.dram_tensor("out", [n, dim], f32, kind="ExternalOutput").ap()}
    with tile.TileContext(nc) as tc:
        for _rep in range(repeats):
            with ExitStack() as ctx:
                build_head_kernel(ctx, tc, outs, ins, n=n, dim=dim, d=d)
    nc.compile()
    _CACHE[key] = nc
    return nc


def run_on_hw(in_maps, trace=False, **kwargs):
    from concourse.bass_utils import run_bass_kernel_spmd

    nc = _build_and_compile(num_cores=len(in_maps))
    return run_bass_kernel_spmd(nc, in_maps, core_ids=list(range(len(in_maps))),
                                trace=trace, **kwargs)


def kernel(Q, K, V, Wq, Wk, Wv, Wo):
    in_maps = make_in_maps(np.asarray(Q), np.asarray(K), np.asarray(V),
                           np.asarray(Wq), np.asarray(Wk), np.asarray(Wv),
                           np.asarray(Wo))
    res = run_on_hw(in_maps)
    out = np.zeros((N, DIM), dtype=np.float64)
    for r in res.results:
        out += r["out"].astype(np.float64)
    return out.astype(np.float32)


if __name__ == "__main__":
    from concourse.timeline_sim import TimelineSim
    nc = _build_and_compile(num_cores=1)
    print("makespan:", TimelineSim(nc).simulate())
